# revision 53
# baseline (speedup 1.0000x reference)
"""Distributed Trainium2 kernel for fused multi-head attention
(QKV projection + RoPE + softmax attention + output projection).

Problem: x[2,2048,1024], Wqkv[1024,3072], bqkv[3072], Wproj[1024,1024], bproj[1024]
NUM_HEADS=16, head_dim=64, non-causal, RoPE (half-split), scale hd^-0.5.

Sharding over 8 NeuronCores: 2-way batch x 4-way head-group tensor parallel.
Core c: batch b=c//4, head group g=c%4 (heads 4g..4g+3).

Design (evolved from the session-1 baseline via trace iteration):
- QKV projection in bf16 at the N=512 streaming roofline; RoPE halves fire
  on DVE as soon as their raw columns exist, overlapped with the remaining
  QKV matmuls; scatter DMAs on sync+gpsimd so ScalarE stays free for evacs.
- Scores keep the K=128 channel-duplication trick (K=64 leaves the HAM
  clock gate cold - measured 1.2GHz through all of attention - and
  row-packed pairs do not actually overlap).
- Softmax exp split ScalarE/DVE per [128,512] tile (Schraudolph fast-exp
  int16-bitcast on DVE); ScalarE stays on the Exp table during attention
  (normalization moved off ScalarE to kill activation-table thrash).
- PV in S^T layout with ones-augmented V (denominator for free); PV
  sub-chains interleaved with the NEXT block's score matmuls at emission
  level so the PE never stalls on the exp engines (PSUM st pool is only
  4 banks deep).
- Normalize fused into the PV evac (DVE tensor_scalar by 1/denom per
  token partition); transpose to [chan, tok] via PE matmuls against the
  identity, two heads col-tiled per pass.
- A2A payload pre-transposed [chan, tok] in 3 token-chunks [1024,512,512];
  receiver runs the token-sharded output projection directly from the A2A
  blocks (no receiver-side transposes); the two earlier chunks' out-proj
  plus keep-warm filler cover the last exposed A2A.
"""
import sys

sys.path.insert(0, "/opt/trn_rl_repo")

import numpy as np
import ml_dtypes

BF16NP = ml_dtypes.bfloat16

N_CORES = 8
B, S, D = 2, 2048, 1024
H, HD = 16, 64
HPG = 4            # heads per group
NPAIR = 2          # head pairs per group
TOK = S            # tokens per batch
KT = D // 128      # 8 contraction tiles for D
SK = S // 128      # 16 key tiles
CHUNKS = [(0, 1024), (1024, 512), (1536, 512)]
ROWB = [0, 256, 384]   # out_d row base per chunk (csize/4 rows per chunk)
ROPE_BASE = 10000.0

TRACE = False
LAST_EXEC_NS = None

# Schraudolph fast-exp in bf16 space (DVE offload of part of the softmax
# exp): exp(x*0.0625) ~= bitcast_bf16(int16(x * S_FE + B_FE)); scores are
# 2x (duplicated channels) so the fold is 1/16.
S_FE = 184.6645 * 0.0625
B_FE = 16248.7
# exp tile engine split: counter % EXP_MOD < EXP_ACT -> ScalarE, else DVE
EXP_ACT = 3
EXP_MOD = 5

_CACHE = {}


def _build_nc():
    import concourse.bass as bass  # noqa
    import concourse.bacc as bacc
    import concourse.mybir as mybir
    from concourse import tile

    F32 = mybir.dt.float32
    BF16 = mybir.dt.bfloat16
    I16 = mybir.dt.int16
    AF = mybir.ActivationFunctionType
    ALU = mybir.AluOpType

    nc = bacc.Bacc("TRN2", target_bir_lowering=False, debug=False,
                   num_devices=N_CORES)

    # ---- per-core DRAM parameters, pre-arranged in SBUF layout on host ----
    xT_d = nc.dram_tensor("xT", [128, KT * TOK], BF16, kind="ExternalInput")
    wq_d = nc.dram_tensor("wq", [128, 4 * KT * 128], BF16, kind="ExternalInput")
    wv_d = nc.dram_tensor("wv", [128, KT * HPG * 65], BF16, kind="ExternalInput")
    wvo_d = nc.dram_tensor("wvo", [1, HPG * 65], BF16, kind="ExternalInput")
    cos_d = nc.dram_tensor("cosT", [128, TOK], BF16, kind="ExternalInput")
    sin_d = nc.dram_tensor("sinT", [128, TOK], BF16, kind="ExternalInput")
    bias_d = nc.dram_tensor("biases", [128, 4], F32, kind="ExternalInput")
    ones_b_d = nc.dram_tensor("ones_b", [1, 128], BF16, kind="ExternalInput")
    ident_d = nc.dram_tensor("ident", [128, 128], BF16, kind="ExternalInput")
    wp_d = nc.dram_tensor("wp", [128, KT * D], BF16, kind="ExternalInput")
    bp_d = nc.dram_tensor("bp", [1, D], BF16, kind="ExternalInput")
    out_d = nc.dram_tensor("out", [512, D], BF16, kind="ExternalOutput")

    with tile.TileContext(nc) as tc:
        with tc.tile_pool(name="const", bufs=1) as constp, \
             tc.tile_pool(name="persist", bufs=1) as persist, \
             tc.tile_pool(name="dram", bufs=1, space="DRAM") as dram:

            # consts on the scalar queue: the sync queue's head is the x
            # chunk-0 tiles (QKV critical path)
            ones_b = constp.tile([1, 128], BF16)
            nc.scalar.dma_start(ones_b[:], ones_b_d[:])
            ident = constp.tile([128, 128], BF16)
            nc.scalar.dma_start(ident[:], ident_d[:])
            bias4 = constp.tile([128, 4], F32)
            nc.scalar.dma_start(bias4[:], bias_d[:])
            bias_sb = {nm: bias4[:, i:i + 1]
                       for i, nm in enumerate(("qa", "qb", "ka", "kb"))}

            # RoPE'd per-head q/k tiles, channel-DUPLICATED: rows 0:64 and
            # 64:128 both hold the head's 64 channels, so the scores matmul
            # contracts K=128 (2x redundant, exp scale halved). K=64 leaves
            # the HAM clock gate cold (measured: whole attention at 1.2GHz)
            # and row-packed pairs do NOT overlap; K=128 keeps 2.4 GHz.
            qt = [persist.tile([128, TOK], BF16, name=f"qt{p}")
                  for p in range(HPG)]
            kt_ = [persist.tile([128, TOK], BF16, name=f"ktp{p}")
                   for p in range(HPG)]
            # V (ones-augmented): sk-block at cols sk*260, head h at +h*65
            vaug = persist.tile([128, SK * HPG * 65], BF16)
            # transposed normalized outputs, per (chunk, pair, quad):
            # oT [128 ch (2 heads), 512 toks]
            oT = {}
            for ci, (_, cs) in enumerate(CHUNKS):
                for p in range(NPAIR):
                    for q in range(cs // 512):
                        oT[(ci, p, q)] = persist.tile(
                            [128, 512], BF16, name=f"oT{ci}_{p}_{q}")

            a2a_in = [dram.tile([2048, cs // 8], BF16, name=f"a2a_in{c}")
                      for c, (_, cs) in enumerate(CHUNKS)]
            a2a_out = [dram.tile([2048, cs // 8], BF16, name=f"a2a_out{c}")
                       for c, (_, cs) in enumerate(CHUNKS)]

            # ---------------- phase 1+2: QKV projection + RoPE ----------
            xv_ctx = tc.tile_pool(name="xv", bufs=1)
            xv = xv_ctx.__enter__()
            with tc.tile_pool(name="raw", bufs=1) as rawp:
                raw = {nm: rawp.tile([128, TOK], BF16, name=f"raw_{nm}")
                       for nm in ("qa", "qb", "ka", "kb")}
                cosT = rawp.tile([128, TOK], BF16)
                sinT = rawp.tile([128, TOK], BF16)

                with tc.tile_pool(name="xw", bufs=1) as xw, \
                     tc.tile_pool(name="qk_ps", bufs=4, space="PSUM") as qk_ps, \
                     tc.tile_pool(name="rope", bufs=2) as ropep:

                    # x chunk-0 tiles first on the two fast HWDGE queues
                    # (QKV's critical path); weights on the gpsimd SWDGE
                    xt = [xv.tile([128, TOK], BF16, name=f"xt{k}")
                          for k in range(KT)]
                    for k in range(KT):
                        eng = nc.sync if k % 2 == 0 else nc.scalar
                        eng.dma_start(xt[k][:, 0:512],
                                      xT_d[:, k * TOK:k * TOK + 512])
                    wall = xw.tile([128, 4 * KT * 128], BF16)
                    nc.gpsimd.dma_start(wall[:], wq_d[:])
                    for quart in range(1, 4):
                        engs = [nc.sync, nc.scalar, nc.gpsimd]
                        for k in range(KT):
                            engs[k % 3].dma_start(
                                xt[k][:, quart * 512:(quart + 1) * 512],
                                xT_d[:, k * TOK + quart * 512:
                                     k * TOK + (quart + 1) * 512])
                    # RoPE tables AFTER all x tiles (they'd delay QKV on
                    # the scalar queue; they're needed only at ~60us)
                    nc.scalar.dma_start(cosT[:], cos_d[:])
                    nc.scalar.dma_start(sinT[:], sin_d[:])
                    # PE warmup during the x-DMA ramp keeps the HAM clock
                    # gate at 2.4 GHz when real work arrives
                    with tc.tile_pool(name="warm_ps", bufs=1,
                                      space="PSUM") as warm_ps:
                        wps = warm_ps.tile([128, 512], F32)
                        for i in range(96):
                            nc.tensor.matmul(wps[:, 0:128], ident[:], ident[:],
                                             start=True, stop=True)
                    w_sb = {nm: wall[:, i * KT * 128:(i + 1) * KT * 128]
                            for i, nm in enumerate(("qa", "qb", "ka", "kb"))}
                    wv_sb = xv.tile([128, KT * HPG * 65], BF16)
                    nc.gpsimd.dma_start(wv_sb[:], wv_d[:])
                    wv_ones = xv.tile([1, HPG * 65], BF16)
                    nc.gpsimd.dma_start(wv_ones[:], wvo_d[:])

                    def rope_half(pref, half):
                        # RoPE on DVE, scatter on sync+gpsimd so the scalar
                        # queue stays free for the QKV/V-proj PSUM evacs
                        h0, h1 = half * 1024, (half + 1) * 1024
                        a_r, b_r = raw[pref + "a"], raw[pref + "b"]
                        dst = qt if pref == "q" else kt_
                        m1 = ropep.tile([128, 1024], BF16, name="m1", tag="m1")
                        nc.vector.tensor_tensor(m1[:], a_r[:, h0:h1],
                                                cosT[:, h0:h1], ALU.mult)
                        m2 = ropep.tile([128, 1024], BF16, name="m2", tag="m2")
                        nc.vector.tensor_tensor(m2[:], b_r[:, h0:h1],
                                                sinT[:, h0:h1], ALU.mult)
                        ar = ropep.tile([128, 1024], BF16, name="ar", tag="ar")
                        nc.vector.tensor_tensor(ar[:], m1[:], m2[:],
                                                ALU.subtract)
                        m3 = ropep.tile([128, 1024], BF16, name="m3", tag="m1")
                        nc.vector.tensor_tensor(m3[:], b_r[:, h0:h1],
                                                cosT[:, h0:h1], ALU.mult)
                        m4 = ropep.tile([128, 1024], BF16, name="m4", tag="m2")
                        nc.vector.tensor_tensor(m4[:], a_r[:, h0:h1],
                                                sinT[:, h0:h1], ALU.mult)
                        br = ropep.tile([128, 1024], BF16, name="br", tag="br")
                        nc.vector.tensor_tensor(br[:], m3[:], m4[:], ALU.add)
                        for j in range(HPG):       # local head j
                            nc.sync.dma_start(dst[j][0:32, h0:h1],
                                              ar[j * 32:(j + 1) * 32, :])
                            nc.sync.dma_start(dst[j][32:64, h0:h1],
                                              br[j * 32:(j + 1) * 32, :])
                            # duplicate rows 0:64 -> 64:128 (K=128 trick)
                            nc.gpsimd.dma_start(dst[j][64:96, h0:h1],
                                                ar[j * 32:(j + 1) * 32, :])
                            nc.gpsimd.dma_start(dst[j][96:128, h0:h1],
                                                br[j * 32:(j + 1) * 32, :])

                    # token-chunk loop: QKV matmuls + evac+bias (on ACT).
                    # RoPE strictly after (interleaving it into the chunk
                    # loop measured 13us SLOWER - queue contention).
                    for ch in range(4):
                        c0, c1 = ch * 512, (ch + 1) * 512
                        for nm in ("qa", "qb", "ka", "kb"):
                            ps = qk_ps.tile([128, 512], F32, name="qkps",
                                            tag="qkps")
                            for k in range(KT):
                                nc.tensor.matmul(
                                    ps[:],
                                    w_sb[nm][:, k * 128:(k + 1) * 128],
                                    xt[k][:, c0:c1],
                                    start=(k == 0), stop=(k == KT - 1))
                            nc.scalar.add(
                                raw[nm][:, c0:c1], ps[:], bias_sb[nm][:])
                    for pref in ("k", "q"):
                        for half in range(2):
                            rope_half(pref, half)

            # (V projection moved into phase 3, interleaved with the first
            # block's scores; xv stays open so xt/wv remain valid)

            # ---------------- phase 3: attention (sw-pipelined) ---------
            wpp_ctx = tc.tile_pool(name="wppool", bufs=1)
            wpp = wpp_ctx.__enter__()
            wp_sb = wpp.tile([128, KT * D], BF16)
            for quart in range(4):
                nc.gpsimd.dma_start(
                    wp_sb[:, quart * 2 * D:(quart + 1) * 2 * D],
                    wp_d[:, quart * 2 * D:(quart + 1) * 2 * D])
            bp_sb = wpp.tile([1, D], BF16)
            nc.gpsimd.dma_start(bp_sb[:], bp_d[:])

            # Rank-sync warm-up AllGather: absorbs PJRT dispatch skew while
            # the QKV phase runs, so the AllToAlls pay only wire time.
            sync_in = dram.tile([8, 16], BF16, name="sync_in")
            sync_out = dram.tile([64, 16], BF16, name="sync_out")
            nc.sync.dma_start(sync_in[:], ones_b_d[:].rearrange(
                "o (p n) -> (o p) n", p=8))
            nc.gpsimd.collective_compute(
                "AllGather", ALU.bypass,
                replica_groups=[[0, 1, 2, 3, 4, 5, 6, 7]],
                ins=[sync_in.opt()], outs=[sync_out.opt()])

            BLOCKS = [(ci, h) for ci in range(len(CHUNKS))
                      for h in range(HPG)]
            est_map = {}
            expctr = [0]

            with tc.tile_pool(name="st_ps", bufs=4, space="PSUM") as st_ps, \
                 tc.tile_pool(name="esb", bufs=82) as esb, \
                 tc.tile_pool(name="olp", bufs=10) as olp, \
                 tc.tile_pool(name="nrm", bufs=12) as nrmp:

                def emit_scores_sk(ci, h, sk):
                    base, csize = CHUNKS[ci]
                    for n in range(csize // 512):
                        c0 = base + n * 512
                        st = st_ps.tile([128, 512], F32, name="st", tag="st")
                        nc.tensor.matmul(
                            st[:],
                            kt_[h][:, sk * 128:(sk + 1) * 128],
                            qt[h][:, c0:c0 + 512],
                            start=True, stop=True)
                        # scores are 2x (duplicated channels): scale 1/16
                        ec = expctr[0]
                        expctr[0] += 1
                        if ec % EXP_MOD < EXP_ACT:
                            est = esb.tile([128, 512], BF16,
                                           name=f"e{ci}_{h}_{sk}_{n}",
                                           tag="est")
                            nc.scalar.activation(
                                est[:], st[:], AF.Exp,
                                bias=0.0, scale=0.0625)
                            est_map[(ci, h, sk, n)] = est[:]
                        else:
                            esti = esb.tile([128, 512], I16,
                                            name=f"e{ci}_{h}_{sk}_{n}",
                                            tag="est")
                            nc.vector.tensor_scalar(
                                esti[:], st[:], S_FE, B_FE,
                                ALU.mult, ALU.add)
                            est_map[(ci, h, sk, n)] = esti[:].bitcast(BF16)

                # V projection, interleaved with block-0 scores over its
                # last chains: the exp engines drain block 0 while the PE
                # does V work, so PV(0,0) starts right after V instead of
                # ~13us later. Evac on ScalarE (GpSimd busy with scatter).
                sk0 = 0
                with tc.tile_pool(name="v_ps", bufs=3,
                                  space="PSUM") as v_ps:
                    for skv in range(SK):
                        ps = v_ps.tile([128, HPG * 65], F32,
                                       name="vps", tag="vps")
                        for k in range(KT):
                            nc.tensor.matmul(
                                ps[:],
                                xt[k][:, skv * 128:(skv + 1) * 128],
                                wv_sb[:, k * (HPG * 65):
                                      (k + 1) * (HPG * 65)],
                                start=(k == 0), stop=False)
                        nc.tensor.matmul(ps[:], ones_b[:], wv_ones[:],
                                         start=False, stop=True)
                        nc.scalar.mul(
                            vaug[:, skv * (HPG * 65):
                                 (skv + 1) * (HPG * 65)], ps[:], 1.0)
                        if skv >= 9:
                            for _ in range(3):
                                if sk0 < SK:
                                    emit_scores_sk(*BLOCKS[0], sk0)
                                    sk0 += 1
                while sk0 < SK:
                    emit_scores_sk(*BLOCKS[0], sk0)
                    sk0 += 1
                o_ctx = tc.tile_pool(name="o_ps", bufs=2, space="PSUM")
                o_ps = o_ctx.__enter__()
                ot_ctx = tc.tile_pool(name="ot_ps", bufs=2, space="PSUM")
                ot_ps = ot_ctx.__enter__()

                def emit_pv_subchain(ci, h, q, s4, ops):
                    sub = q * 4 + s4
                    for sk in range(SK):
                        nc.tensor.matmul(
                            ops[:, s4 * 65:s4 * 65 + 65],
                            est_map[(ci, h, sk, sub // 4)][
                                :, (sub % 4) * 128:(sub % 4 + 1) * 128],
                            vaug[:, sk * (HPG * 65) + h * 65:
                                 sk * (HPG * 65) + h * 65 + 65],
                            start=(sk == 0), stop=(sk == SK - 1))

                def emit_pv_finish(ci, h, q, ops, out):
                    # recip of the 4 denominators (cols 64, 129, ...)
                    rec4 = nrmp.tile([128, 4], F32, name="rec", tag="rec")
                    nc.vector.reciprocal(rec4[:], ops[:, 64:260:65])
                    # fused evac+normalize: ol[t, c] = ops[t, c]/denom[t]
                    ol = olp.tile([128, 256], BF16,
                                  name=f"ol{ci}_{h}_{q}", tag="ol")
                    for s4 in range(4):
                        nc.vector.tensor_scalar(
                            ol[:, s4 * 64:(s4 + 1) * 64],
                            ops[:, s4 * 65:s4 * 65 + 64],
                            rec4[:, s4:s4 + 1], None, ALU.mult)
                    out[q] = ol

                def emit_trans(ci, p, nrm):
                    # transpose via PE: oT[64j+c, t] = ol[t, c], two heads
                    # col-tiled concurrently against the shared identity
                    base, csize = CHUNKS[ci]
                    for q in range(csize // 512):
                        otp = ot_ps.tile([128, 512], F32,
                                         name=f"otp{ci}_{p}_{q}", tag="otp")
                        for s4 in range(4):
                            for j in range(2):
                                h = 2 * p + j
                                ol = nrm[h][q]
                                nc.tensor.matmul(
                                    otp[j * 64:j * 64 + 64,
                                        s4 * 128:(s4 + 1) * 128],
                                    ol[:, s4 * 64:s4 * 64 + 64],
                                    ident[:],
                                    start=True, stop=True)
                        nc.vector.tensor_copy(oT[(ci, p, q)][:], otp[:])

                def emit_a2a(ci):
                    base, csize = CHUNKS[ci]
                    shard = csize // 8
                    for s in range(8):
                        for p in range(NPAIR):
                            if csize == 1024:
                                src = oT[(ci, p, s // 4)][
                                    :, (s % 4) * 128:(s % 4) * 128 + 128]
                            else:
                                src = oT[(ci, p, 0)][:, s * 64:(s + 1) * 64]
                            eng = nc.sync if p == 0 else nc.gpsimd
                            eng.dma_start(
                                a2a_in[ci][s * 256 + p * 128:
                                           s * 256 + (p + 1) * 128, :],
                                src)
                    nc.gpsimd.collective_compute(
                        "AllToAll", ALU.bypass,
                        replica_groups=[[0, 1, 2, 3, 4, 5, 6, 7]],
                        ins=[a2a_in[ci].opt()], outs=[a2a_out[ci].opt()])

                nrm = {}
                for i, (ci, h) in enumerate(BLOCKS):
                    nxt = BLOCKS[i + 1] if i + 1 < len(BLOCKS) else None
                    base, csize = CHUNKS[ci]
                    nsub = csize // 128
                    skper = -(-SK // nsub)   # next-block sk groups per chain
                    sknext = 0
                    olt = {}
                    for q in range(csize // 512):
                        ops = o_ps.tile([128, 260], F32,
                                        name=f"ops{ci}_{h}_{q}", tag="ops")
                        for s4 in range(4):
                            emit_pv_subchain(ci, h, q, s4, ops)
                            # interleave next block's scores between chains
                            # so exp (ACT/DVE) stays fed without PE stalls
                            if nxt:
                                for _ in range(skper):
                                    if sknext < SK:
                                        emit_scores_sk(*nxt, sknext)
                                        sknext += 1
                        emit_pv_finish(ci, h, q, ops, olt)
                    if nxt:
                        while sknext < SK:
                            emit_scores_sk(*nxt, sknext)
                            sknext += 1
                    nrm[h] = olt
                    for sk in range(SK):
                        for n in range(csize // 512):
                            est_map.pop((ci, h, sk, n), None)
                    if h % 2 == 1:
                        emit_trans(ci, h // 2, nrm)
                        nrm = {}
                    if h == HPG - 1:
                        emit_a2a(ci)
                ot_ctx.__exit__(None, None, None)
                o_ctx.__exit__(None, None, None)

            # ---------------- phase 4: output projection ----------------
            # Receiver holds [1024 ch, shard toks] blocks; out-proj directly
            # accumulates over the 8 channel k-tiles. Chunks 0,1 overlap the
            # exposed A2A of chunk 2.
            with tc.tile_pool(name="oin", bufs=24) as oinp, \
                 tc.tile_pool(name="op_ps", bufs=3, space="PSUM") as op_ps, \
                 tc.tile_pool(name="warm2", bufs=1, space="PSUM") as w2p, \
                 tc.tile_pool(name="osb", bufs=6) as osb:
                wtile = w2p.tile([128, 512], F32)

                def emit_outproj(ci):
                    base, csize = CHUNKS[ci]
                    shard = csize // 8
                    # per-chunk DMA queue so one chunk's loads don't
                    # head-of-line block the next chunk's
                    ldeng = [nc.sync, nc.scalar, nc.gpsimd][ci]
                    # pack batches along the lhsT free dim: shard=64 chunks
                    # get both batches' tokens in ONE M=128 chain (the two
                    # out_d row ranges are contiguous), halving the exposed
                    # tail matmuls; shard=128 runs one chain per batch.
                    bper = 128 // shard
                    for grp in range(2 // bper):
                        oin = []
                        for k in range(KT):
                            t = oinp.tile([128, 128], BF16,
                                          name=f"oin{ci}_{grp}_{k}",
                                          tag="oin")
                            for bb in range(bper):
                                beta = grp * bper + bb
                                r0 = (4 * beta + k // 2) * 256 + \
                                    (k % 2) * 128
                                ldeng.dma_start(
                                    t[:, bb * shard:(bb + 1) * shard],
                                    a2a_out[ci][r0:r0 + 128, :])
                            oin.append(t)
                        for ncol in range(2):
                            ps = op_ps.tile([128, 512], F32, name="oppsum",
                                            tag="oppsum")
                            for k in range(KT):
                                nc.tensor.matmul(
                                    ps[:],
                                    oin[k][:],
                                    wp_sb[:, k * D + ncol * 512:
                                          k * D + (ncol + 1) * 512],
                                    start=(k == 0), stop=False)
                            nc.tensor.matmul(
                                ps[:], ones_b[0:1, 0:128],
                                bp_sb[:, ncol * 512:(ncol + 1) * 512],
                                start=False, stop=True)
                            ob = osb.tile([128, 512], BF16, name="ob",
                                          tag="ob")
                            nc.vector.tensor_copy(ob[:], ps[:])
                            for hh in range(2):
                                eng = nc.sync if hh == 0 else nc.scalar
                                eng.dma_start(
                                    out_d[ROWB[ci] + grp * 128:
                                          ROWB[ci] + (grp + 1) * 128,
                                          ncol * 512 + hh * 256:
                                          ncol * 512 + (hh + 1) * 256],
                                    ob[:, hh * 256:(hh + 1) * 256])

                emit_outproj(0)
                emit_outproj(1)
                # keep-warm filler while waiting for the last A2A
                for i in range(24):
                    nc.tensor.matmul(wtile[:, 0:128], ident[:], ident[:],
                                     start=True, stop=True)
                emit_outproj(2)
            wpp_ctx.__exit__(None, None, None)
            xv_ctx.__exit__(None, None, None)
    nc.compile()
    return nc


def _prepare_inputs(x, Wqkv, bqkv, Wproj, bproj):
    """Build the 8 per-core input maps (host-side sharding only)."""
    W3 = Wqkv.reshape(D, 3, H, HD)
    b3 = bqkv.reshape(3, H, HD)

    def to_sbuf_layout(w):  # [D, N] -> [128, KT*N]
        n = w.shape[1]
        return np.ascontiguousarray(
            w.reshape(KT, 128, n).transpose(1, 0, 2).reshape(128, KT * n))

    # RoPE tables, stacked layout [128, TOK]: row j*32+c -> cos(ang[pos, c])
    inv = (1.0 / (ROPE_BASE ** (np.arange(0, HD, 2, dtype=np.float64) / HD)))
    ang = np.arange(TOK, dtype=np.float64)[:, None] * inv[None, :]  # [TOK, 32]
    cosT = np.tile(np.cos(ang).T.astype(np.float32), (4, 1)).astype(BF16NP)
    sinT = np.tile(np.sin(ang).T.astype(np.float32), (4, 1)).astype(BF16NP)

    wp_bf = to_sbuf_layout(Wproj).astype(BF16NP)
    bp_eff = (bqkv[2 * D:3 * D].astype(np.float64) @ Wproj.astype(np.float64)
              + bproj.astype(np.float64)).astype(np.float32)
    bp_bf = bp_eff[None, :].astype(BF16NP)
    ones_b = np.ones((1, 128), BF16NP)
    ident = np.eye(128, dtype=np.float32).astype(BF16NP)

    in_maps = []
    for c in range(N_CORES):
        b, g = divmod(c, 4)
        hs = slice(4 * g, 4 * g + 4)
        xT = to_sbuf_layout(
            np.ascontiguousarray(x[b].T)).astype(BF16NP)  # [128, KT*TOK]

        wq_parts = [
            W3[:, 0, hs, 0:32].reshape(D, 128),
            W3[:, 0, hs, 32:64].reshape(D, 128),
            W3[:, 1, hs, 0:32].reshape(D, 128),
            W3[:, 1, hs, 32:64].reshape(D, 128),
        ]
        wq = np.concatenate(
            [to_sbuf_layout(np.ascontiguousarray(w)) for w in wq_parts],
            axis=1).astype(BF16NP)  # [128, 4*KT*128]

        wv = np.zeros((D, HPG * 65), np.float32)
        wv.reshape(D, HPG, 65)[:, :, 0:64] = W3[:, 2, hs, :]
        wv = to_sbuf_layout(wv).astype(BF16NP)
        wvo = np.zeros((1, HPG * 65), np.float32)
        for j in range(HPG):
            wvo[0, j * 65 + 64] = 1.0
        wvo = wvo.astype(BF16NP)

        biases = np.stack([
            b3[0, hs, 0:32].reshape(128),
            b3[0, hs, 32:64].reshape(128),
            b3[1, hs, 0:32].reshape(128),
            b3[1, hs, 32:64].reshape(128),
        ], axis=1).astype(np.float32)  # [128, 4]

        in_maps.append({
            "xT": xT, "wq": wq, "wv": wv, "wvo": wvo,
            "cosT": cosT, "sinT": sinT, "biases": biases,
            "ones_b": ones_b, "ident": ident,
            "wp": wp_bf, "bp": bp_bf,
        })
    return in_maps


def kernel(x, Wqkv, bqkv, Wproj, bproj):
    global LAST_EXEC_NS
    from concourse.bass_utils import run_bass_kernel_spmd

    if "nc" not in _CACHE:
        _CACHE["nc"] = _build_nc()
    nc = _CACHE["nc"]

    in_maps = _prepare_inputs(
        np.asarray(x, np.float32), np.asarray(Wqkv, np.float32),
        np.asarray(bqkv, np.float32), np.asarray(Wproj, np.float32),
        np.asarray(bproj, np.float32))

    kw = {}
    if TRACE:
        kw["trace"] = True
    res = run_bass_kernel_spmd(nc, in_maps, core_ids=list(range(N_CORES)), **kw)
    LAST_EXEC_NS = res.exec_time_ns

    out = np.empty((B, S, D), np.float32)
    for c in range(N_CORES):
        rr = np.asarray(res.results[c]["out"], BF16NP).astype(np.float32)
        for ci, (base, csize) in enumerate(CHUNKS):
            shard = csize // 8
            for beta in range(B):
                out[beta, base + c * shard:base + (c + 1) * shard] = \
                    rr[ROWB[ci] + beta * shard:ROWB[ci] + (beta + 1) * shard]
    return out
